# revision 21
# baseline (speedup 1.0000x reference)
"""TAGConv (2-layer, K=3) GNN encoder on 8 Trainium2 NeuronCores.

Strategy (graph/data parallel, per sharding hint):
  - Renumber nodes with a degree-balancing snake permutation into tiles of
    128; each of the 8 cores owns NT tiles of destination nodes.
  - Fold gcn_norm into per-node scales: h_next = dis * segsum(dis * h).
  - Per hop: dma_gather edge-source rows (bf16) from node-feature tables
    in local DRAM; segment-sum via PE matmul with an on-the-fly one-hot
    (DVE is_equal against iota); scale by dis (per partition);
    PE-transpose for the per-hop weight matmul.
  - Gather descriptor emission is the bottleneck: round-robin gather
    calls over 4 SWDGE queues (each queue runs on its own Q7 core pair,
    so emission is 4-way parallel).
  - Tables are quartered along stage boundaries; each quarter is
    AllGather'd as soon as its stages finish and copied Shared->local
    behind the remaining compute (random reads from Shared DRAM are
    slow). Gather index ranges == quarters, which also keeps int16
    index reach.
  - The layer-1/layer-2 boundary (relu + W2[0] term + boundary table)
    is folded into layer-1 hop 3 per-stage so the boundary table's
    quarters fire early too.
"""

import math
import numpy as np
import ml_dtypes

import concourse.bass as bass
import concourse.mybir as mybir
import concourse.tile as tile
from concourse import bacc
from concourse.bass_utils import run_bass_kernel_spmd
from concourse.masks import make_identity

P = 128
MAX_RANGE = 32768  # int16 index reach for dma_gather
NSWQ = 4          # SWDGE queues; queue q runs on Q7 core pair q, so
                  # round-robin gather calls emit descriptors 4-way parallel
GMAX_CHUNKS = 16  # chunks per gather call (nidx = GMAX*128)
NQUART = 4        # table quarters (AllGather granularity)
DEBUG_NO_AG = 0  # 1: skip collectives (wrong results)
DEBUG_GATHER_T0 = 0  # 1: always gather from table0 (wrong results)
DEBUG_NO_GATHER = 0  # 1: skip dma_gather entirely (wrong results)
DEBUG_GATHER_ONLY = 0  # 1: gathers only, skip compute consumers (wrong)

BF16 = mybir.dt.bfloat16
F32 = mybir.dt.float32
I16 = mybir.dt.int16
NP_BF16 = ml_dtypes.bfloat16


class Cfg:
    def __init__(self, n, d_in, d_out, k, ncores, stage_tiles=6):
        self.N = n
        self.D = d_in          # feature width (128)
        self.DO = d_out        # output width (16)
        self.K = k             # hops per conv
        self.M = ncores
        block = P * ncores
        self.NPAD = ((n + block - 1) // block) * block
        self.R = self.NPAD // ncores      # rows per core
        self.NT = self.R // P             # dst tiles per core
        self.TG = self.NPAD // P          # global tiles
        self.S = min(stage_tiles, self.NT)
        self.stages = [(i, min(self.S, self.NT - i))
                       for i in range(0, self.NT, self.S)]


# ---------------------------------------------------------------- host prep

def _preprocess(cfg, x, edge_index):
    """Build permutation, per-core edge slot arrays, and scales."""
    N, NPAD, M = cfg.N, cfg.NPAD, cfg.M
    src_o = edge_index[0].astype(np.int64)
    dst_o = edge_index[1].astype(np.int64)
    E = src_o.shape[0]

    deg = np.bincount(dst_o, minlength=N).astype(np.float32)
    dis = np.where(deg > 0, 1.0 / np.sqrt(np.maximum(deg, 1.0)), 0.0)
    dis = dis.astype(np.float32)

    # snake assignment of degree-sorted nodes to TG tiles
    TG = cfg.TG
    order = np.argsort(-deg, kind="stable")
    newid_of_old = np.empty(N, dtype=np.int64)
    tile_fill = np.zeros(TG, dtype=np.int64)
    pos = 0
    fwd = True
    while pos < N:
        m = min(TG, N - pos)
        bins = np.arange(m) if fwd else (TG - 1 - np.arange(m))
        nodes = order[pos:pos + m]
        newid_of_old[nodes] = bins * P + tile_fill[bins]
        tile_fill[bins] += 1
        pos += m
        fwd = not fwd
    assert tile_fill.max() <= P

    old_of_new = np.full(NPAD, -1, dtype=np.int64)
    old_of_new[newid_of_old] = np.arange(N)

    NT, S, R = cfg.NT, cfg.S, cfg.R
    nstage = len(cfg.stages)

    # quarters = groups of stages (AllGather granularity AND gather ranges)
    Q = min(NQUART, nstage)
    groups = [list(g) for g in np.array_split(np.arange(nstage), Q)]
    qt_sz = [int(sum(cfg.stages[si][1] for si in g)) for g in groups]
    qt_off = np.concatenate([[0], np.cumsum(qt_sz)]).astype(np.int64)
    qot = np.empty(NT, dtype=np.int64)  # quarter of local tile
    for q, g in enumerate(groups):
        for si in g:
            t0, ns = cfg.stages[si]
            qot[t0:t0 + ns] = q
    qg_sz = [M * s * P for s in qt_sz]          # global rows per quarter
    qg_base = np.concatenate([[0], np.cumsum(qg_sz)]).astype(np.int64)
    assert max(qg_sz) <= MAX_RANGE, (qg_sz, "int16 gather reach exceeded")

    # table-row id (gather space) for every snake id
    ids = np.arange(NPAD, dtype=np.int64)
    c_of = ids // R
    j_of = (ids % R) // P
    p_of = ids % P
    q_of = qot[j_of]
    trow_of_new = (qg_base[q_of] + c_of * (np.array(qt_sz)[q_of] * P)
                   + (j_of - qt_off[q_of]) * P + p_of)

    src = newid_of_old[src_o]
    dst = newid_of_old[dst_o]
    tile_g = dst >> 7
    rng = qot[(src % R) // P]        # quarter of the source node
    src_t = trow_of_new[src]         # gather-space row of the source

    # per-(tile, range) counts -> global fixed chunk counts C[r]
    NR = Q
    key = tile_g * NR + rng
    cnt = np.bincount(key, minlength=TG * NR).reshape(TG, NR)
    C = [int(math.ceil(cnt[:, r].max() / P)) for r in range(NR)]
    C = [max(c, 1) for c in C]
    CT = sum(C)
    c_off = np.concatenate([[0], np.cumsum(C)])  # chunk offset of range r

    # column index of chunk (s, r, tt, k) within a core
    stage_cols = [ns * CT for (_, ns) in cfg.stages]
    stage_base = np.concatenate([[0], np.cumsum(stage_cols)])
    TOTC = NT * CT                       # chunks per core
    TOTS = TOTC * P                      # slots per core

    # slot base for every (global tile, range)
    t_loc = np.arange(TG) % NT
    s_idx = t_loc // S
    tt = t_loc % S
    ns_of = np.array([cfg.stages[i][1] for i in s_idx])
    r_off = np.zeros((TG, NR), dtype=np.int64)
    for r in range(NR):
        r_off[:, r] = ns_of * c_off[r]
    colbase = (stage_base[s_idx][:, None] + r_off
               + (tt[:, None] * np.array(C)[None, :] + 0))
    # colbase[t, r] = first chunk column (within core) of (t, r)
    slotbase = colbase * P

    # order edges by (tile, range), cumcount within group
    eorder = np.argsort(key, kind="stable")
    key_s = key[eorder]
    grp_start = np.zeros(TG * NR + 1, dtype=np.int64)
    np.cumsum(np.bincount(key_s, minlength=TG * NR), out=grp_start[1:])
    within = np.arange(E, dtype=np.int64) - grp_start[key_s]
    slot_in_core = slotbase.reshape(-1)[key_s] + within
    core_of = (tile_g[eorder]) // NT

    # fill per-core slot arrays
    srcs_slots = np.zeros((M, TOTS), dtype=np.int16)
    dstl_slots = np.full((M, TOTS), -1.0, dtype=np.float32)
    src_local = (src_t - qg_base[rng]).astype(np.int16)
    flat = core_of * TOTS + slot_in_core
    srcs_flat = srcs_slots.reshape(-1)
    dstl_flat = dstl_slots.reshape(-1)
    srcs_flat[flat] = src_local[eorder]
    dstl_flat[flat] = (dst[eorder] & 127).astype(np.float32)

    # wrap srcs: slot j -> [j%16, j//16], tiled over 8 gpsimd cores
    srcs_in = np.empty((M, P, TOTS // 16), dtype=np.int16)
    dstl_in = np.empty((M, P, TOTC), dtype=NP_BF16)
    for c in range(M):
        w16 = srcs_slots[c].reshape(-1, 16).T       # [16, TOTS/16]
        srcs_in[c] = np.tile(w16, (8, 1))
        dstl_in[c] = dstl_slots[c].reshape(-1, P).T.astype(NP_BF16)

    # permuted, padded per-node data (snake order)
    x_pad = np.zeros((NPAD, cfg.D), dtype=np.float32)
    x_pad[newid_of_old] = x
    dis_pad = np.zeros(NPAD, dtype=np.float32)
    dis_pad[newid_of_old] = dis

    # initial gather table in gather-space row order
    table0 = np.zeros((NPAD, cfg.D), dtype=NP_BF16)
    table0[trow_of_new] = (dis_pad[:, None] * x_pad).astype(NP_BF16)
    xT = np.ascontiguousarray(
        x_pad.reshape(M, NT, P, cfg.D).transpose(0, 1, 3, 2)).astype(NP_BF16)
    dis_col = np.ascontiguousarray(
        dis_pad.reshape(M, NT, P).transpose(0, 2, 1)).astype(np.float32)

    iota = np.broadcast_to(np.arange(P, dtype=np.float32), (P, P))
    iota = np.ascontiguousarray(iota).astype(NP_BF16)

    meta = dict(C=C, CT=CT, TOTC=TOTC, TOTS=TOTS, NR=NR,
                stage_base=stage_base, c_off=c_off,
                groups=groups, qt_sz=qt_sz, qt_off=qt_off,
                qg_sz=qg_sz, qg_base=qg_base,
                old_of_new=old_of_new)
    data = dict(table0=table0, xT=xT, dis_col=dis_col,
                srcs=srcs_in, dstl=dstl_in, iota=iota)
    return meta, data


# ---------------------------------------------------------------- device

def _build_program(cfg, meta):
    N, D, DO, K, M = cfg.N, cfg.D, cfg.DO, cfg.K, cfg.M
    NPAD, R, NT = cfg.NPAD, cfg.R, cfg.NT
    NR = meta["NR"]
    C, CT = meta["C"], meta["CT"]
    stage_base, c_off = meta["stage_base"], meta["c_off"]
    TOTC, TOTS = meta["TOTC"], meta["TOTS"]
    groups, qt_sz, qt_off = meta["groups"], meta["qt_sz"], meta["qt_off"]
    qg_sz, qg_base = meta["qg_sz"], meta["qg_base"]
    stages = cfg.stages
    # last stage index of each quarter group
    q_last_stage = {g[-1]: q for q, g in enumerate(groups)}

    nc = bacc.Bacc("TRN2", target_bir_lowering=False, debug=False,
                   num_devices=M, num_swdge_queues=NSWQ)

    table0_d = nc.dram_tensor("table0", [NPAD, D], BF16, kind="ExternalInput")
    xT_d = nc.dram_tensor("xT", [NT, D, P], BF16, kind="ExternalInput")
    srcs_d = nc.dram_tensor("srcs", [P, TOTS // 16], I16, kind="ExternalInput")
    dstl_d = nc.dram_tensor("dstl", [P, TOTC], BF16, kind="ExternalInput")
    iota_d = nc.dram_tensor("iota", [P, P], BF16, kind="ExternalInput")
    discol_d = nc.dram_tensor("discol", [P, NT], F32, kind="ExternalInput")
    w1_d = nc.dram_tensor("w1", [D, (K + 1) * D], BF16, kind="ExternalInput")
    w2_d = nc.dram_tensor("w2", [D, (K + 1) * DO], BF16, kind="ExternalInput")
    b1_d = nc.dram_tensor("b1", [P, D], F32, kind="ExternalInput")
    b2_d = nc.dram_tensor("b2", [P, DO], F32, kind="ExternalInput")
    out_d = nc.dram_tensor("out", [R, DO], F32, kind="ExternalOutput")

    rg = [list(range(M))]

    with tile.TileContext(nc) as tc:
        with (
            tc.tile_pool(name="const", bufs=1) as cpool,
            tc.tile_pool(name="acc", bufs=1) as apool,
            tc.tile_pool(name="gb", bufs=2) as gpool,
            tc.tile_pool(name="oh", bufs=2) as ohpool,
            tc.tile_pool(name="ev", bufs=3) as evpool,
            tc.tile_pool(name="gst", bufs=2) as gstpool,
            tc.tile_pool(name="ps", bufs=2, space="PSUM") as pspool,
            tc.tile_pool(name="pt", bufs=2, space="PSUM") as ptpool,
            tc.tile_pool(name="pw", bufs=2, space="PSUM") as pwpool,
            tc.tile_pool(name="dram", bufs=1, space="DRAM") as dram,
        ):
            # ---- persistent loads
            srcs_sb = cpool.tile([P, TOTS // 16], I16, tag="srcs")
            nc.sync.dma_start(srcs_sb[:], srcs_d[:])
            dstl_sb = cpool.tile([P, TOTC], BF16, tag="dstl")
            nc.sync.dma_start(dstl_sb[:], dstl_d[:])
            iota_sb = cpool.tile([P, P], BF16, tag="iota")
            nc.sync.dma_start(iota_sb[:], iota_d[:])
            discol_sb = cpool.tile([P, NT], F32, tag="discol")
            nc.sync.dma_start(discol_sb[:], discol_d[:])
            w1_sb = cpool.tile([D, (K + 1) * D], BF16, tag="w1")
            nc.sync.dma_start(w1_sb[:], w1_d[:])
            w2_sb = cpool.tile([D, (K + 1) * DO], BF16, tag="w2")
            nc.sync.dma_start(w2_sb[:], w2_d[:])
            b1_sb = cpool.tile([P, D], F32, tag="b1")
            nc.sync.dma_start(b1_sb[:], b1_d[:])
            b2_sb = cpool.tile([P, DO], F32, tag="b2")
            nc.sync.dma_start(b2_sb[:], b2_d[:])
            ident = cpool.tile([P, P], F32, tag="ident")
            make_identity(nc, ident[:])

            out1_sb = apool.tile([P, NT * D], F32, tag="out1")
            out2_sb = apool.tile([P, NT * DO], F32, tag="out2")

            table0_parts = [
                table0_d[int(qg_base[r]):int(qg_base[r]) + qg_sz[r], :]
                for r in range(NR)]

            def w_slice(layer, k):
                if layer == 1:
                    return w1_sb[:, k * D:(k + 1) * D]
                return w2_sb[:, k * DO:(k + 1) * DO]

            def w_accum(layer, t, k, hT):
                """matmul h_T @ W[k] and accumulate into the out buffer."""
                do = D if layer == 1 else DO
                acc = out1_sb if layer == 1 else out2_sb
                pw = pwpool.tile([P, do], F32, tag=f"pw{layer}")
                nc.tensor.matmul(pw[:], lhsT=hT, rhs=w_slice(layer, k),
                                 start=True, stop=True)
                sl = acc[:, t * do:(t + 1) * do]
                if k == 0:
                    nc.vector.tensor_copy(sl, pw[:])
                else:
                    nc.vector.tensor_add(sl, sl, pw[:])

            self_q = [0]  # round-robin gather queue counter

            def hop(layer, k, parts, write_table, boundary=False):
                """One propagation hop.

                parts: per-range gather source APs (local DRAM).
                write_table: produce the next table (quartered AG + local
                  copy); returns list of local quarter tiles, else None.
                boundary: this is layer-1 hop K -- also do relu+bias, the
                  layer-2 k=0 term, and write the boundary table rows
                  dis*relu(out1+b1) instead of dis*h.
                """
                write_table = write_table and not DEBUG_GATHER_ONLY
                tag = "b" if boundary else f"{layer}_{k}"
                if write_table:
                    shard = dram.tile([R, D], BF16, tag=f"shard{tag}")
                    shard_v = shard[:].rearrange("(t p) d -> t p d", p=P)
                    fulls, locs = [], []
                    for q in range(NR):
                        fulls.append(dram.tile(
                            [qg_sz[q], D], BF16, tag=f"table{tag}_q{q}",
                            name=f"table{tag}_q{q}", addr_space="Shared"))
                        locs.append(dram.tile(
                            [qg_sz[q], D], BF16, tag=f"tableL{tag}_q{q}",
                            name=f"tableL{tag}_q{q}"))

                GMAX = GMAX_CHUNKS
                for si, (t0, ns) in enumerate(stages):
                    # gathers for this stage, one buffer per range
                    gbufs = []
                    for r in range(NR):
                        nch = ns * C[r]
                        gb = gpool.tile([P, cfg.S * C[r] * P], BF16,
                                        tag=f"gb{r}")
                        colb = stage_base[si] + ns * c_off[r]
                        for g0 in range(0, nch, GMAX):
                            if DEBUG_NO_GATHER:
                                break
                            gn = min(GMAX, nch - g0)
                            nidx = gn * P
                            cb = colb + g0
                            nc.gpsimd.dma_gather(
                                gb[:, g0 * D:(g0 + gn) * D].rearrange(
                                    "p (c e) -> p c e", e=D),
                                parts[r],
                                srcs_sb[:, cb * 8:cb * 8 + nidx // 16],
                                nidx, nidx, D,
                                single_packet=(gn <= 8),
                                queue_num=self_q[0] % NSWQ,
                            )
                            self_q[0] += 1
                        gbufs.append(gb)

                    if DEBUG_GATHER_ONLY:
                        continue

                    if write_table:
                        gstage = gstpool.tile([P, cfg.S * D], BF16, tag="gst")

                    for tt in range(ns):
                        t = t0 + tt
                        ps = pspool.tile([P, D], F32, tag="ps")
                        ci = 0
                        for r in range(NR):
                            colb = stage_base[si] + ns * c_off[r] + tt * C[r]
                            oh = ohpool.tile([P, C[r] * P], BF16,
                                             tag=f"oh{r}")
                            nc.vector.tensor_tensor(
                                out=oh[:].rearrange("p (c j) -> p c j", j=P),
                                in0=dstl_sb[:, colb:colb + C[r]][:, :, None]
                                    .broadcast_to([P, C[r], P]),
                                in1=iota_sb[:][:, None, :]
                                    .broadcast_to([P, C[r], P]),
                                op=mybir.AluOpType.is_equal,
                            )
                            for kk in range(C[r]):
                                gsl = gbufs[r][:, (tt * C[r] + kk) * D:
                                               (tt * C[r] + kk + 1) * D]
                                nc.tensor.matmul(
                                    ps[:],
                                    lhsT=oh[:, kk * P:(kk + 1) * P],
                                    rhs=gsl,
                                    start=(ci == 0), stop=(ci == CT - 1),
                                )
                                ci += 1
                        # evacuate: h_row = dis * segsum
                        hrow = evpool.tile([P, D], F32, tag="hrow")
                        nc.vector.tensor_scalar_mul(
                            hrow[:], ps[:], discol_sb[:, t:t + 1])
                        if write_table and not boundary:
                            nc.vector.tensor_scalar_mul(
                                gstage[:, tt * D:(tt + 1) * D],
                                hrow[:], discol_sb[:, t:t + 1])
                        pt = ptpool.tile([P, D], F32, tag="pt")
                        nc.tensor.transpose(pt[:], hrow[:], ident[:])
                        hT = evpool.tile([P, D], BF16, tag="hT")
                        nc.vector.tensor_copy(hT[:], pt[:])
                        w_accum(layer, t, k, hT[:])

                        if boundary:
                            # relu(out1[t] + b1) -> layer2 k=0 + table_b
                            sl = out1_sb[:, t * D:(t + 1) * D]
                            h0 = evpool.tile([P, D], F32, tag="h0")
                            nc.vector.tensor_tensor(
                                out=h0[:], in0=sl, in1=b1_sb[:],
                                op=mybir.AluOpType.add)
                            nc.vector.tensor_scalar_max(h0[:], h0[:], 0.0)
                            if write_table:
                                nc.vector.tensor_scalar_mul(
                                    gstage[:, tt * D:(tt + 1) * D],
                                    h0[:], discol_sb[:, t:t + 1])
                            pt2 = ptpool.tile([P, D], F32, tag="pt")
                            nc.tensor.transpose(pt2[:], h0[:], ident[:])
                            h0T = evpool.tile([P, D], BF16, tag="hT")
                            nc.vector.tensor_copy(h0T[:], pt2[:])
                            w_accum(2, t, 0, h0T[:])

                    if write_table:
                        nc.sync.dma_start(
                            shard_v[t0:t0 + ns].rearrange(
                                "t p d -> p t d"),
                            gstage[:, :ns * D].rearrange(
                                "p (t d) -> p t d", d=D))
                        if si in q_last_stage:
                            q = q_last_stage[si]
                            ra = int(qt_off[q]) * P
                            rb = ra + qt_sz[q] * P
                            nc.gpsimd.collective_compute(
                                "AllGather", mybir.AluOpType.bypass,
                                replica_groups=rg,
                                ins=[shard[ra:rb, :].opt()],
                                outs=[fulls[q].opt()])
                            nc.sync.dma_start(locs[q][:], fulls[q][:])

                if write_table:
                    return locs
                return None

            # ---- layer 1, k=0 term: x @ W1[0]
            for t in range(NT):
                xt = evpool.tile([P, P], BF16, tag="xT")
                nc.sync.dma_start(xt[:], xT_d[t])
                w_accum(1, t, 0, xt[:])

            # ---- layer 1 hops (hop K also does the boundary)
            parts = table0_parts
            for k in range(1, K + 1):
                wt = (not DEBUG_NO_AG) and (k < K or True)
                nxt = hop(1, k, parts, write_table=wt, boundary=(k == K))
                if nxt is not None and not DEBUG_GATHER_T0:
                    parts = [loc[:] for loc in nxt]
                elif DEBUG_GATHER_T0 or nxt is None:
                    parts = table0_parts

            # ---- layer 2 hops
            for k in range(1, K + 1):
                wt = (k < K) and not DEBUG_NO_AG
                nxt = hop(2, k, parts, write_table=wt)
                if nxt is not None and not DEBUG_GATHER_T0:
                    parts = [loc[:] for loc in nxt]
                elif DEBUG_GATHER_T0:
                    parts = table0_parts

            # ---- add b2, write out
            if not DEBUG_GATHER_ONLY:
                o2v = out2_sb[:].rearrange("p (t j) -> p t j", j=DO)
                nc.vector.tensor_tensor(
                    out=o2v, in0=o2v,
                    in1=b2_sb[:][:, None, :]
                        .broadcast_to([P, NT, DO]),
                    op=mybir.AluOpType.add)
                nc.sync.dma_start(
                    out_d[:].rearrange("(t p) j -> p t j", p=P),
                    out2_sb[:].rearrange("p (t j) -> p t j", j=DO))

    nc.compile()
    return nc


# ---------------------------------------------------------------- entry

def _run(x, edge_index, W1, b1, W2, b2, ncores=8, trace=False):
    x = np.asarray(x, dtype=np.float32)
    edge_index = np.asarray(edge_index)
    W1 = np.asarray(W1, dtype=np.float32)
    b1 = np.asarray(b1, dtype=np.float32)
    W2 = np.asarray(W2, dtype=np.float32)
    b2 = np.asarray(b2, dtype=np.float32)

    n, d = x.shape
    kp1, _, dh = W1.shape
    _, _, do = W2.shape
    cfg = Cfg(n, d, do, kp1 - 1, ncores)
    meta, data = _preprocess(cfg, x, edge_index)

    w1_flat = np.concatenate(list(W1), axis=1).astype(NP_BF16)
    w2_flat = np.concatenate(list(W2), axis=1).astype(NP_BF16)
    b1_in = np.ascontiguousarray(
        np.broadcast_to(b1.reshape(1, -1), (P, b1.size))).astype(np.float32)
    b2_in = np.ascontiguousarray(
        np.broadcast_to(b2.reshape(1, -1), (P, b2.size))).astype(np.float32)

    in_maps = []
    for c in range(ncores):
        in_maps.append({
            "table0": data["table0"],
            "xT": np.ascontiguousarray(data["xT"][c]),
            "srcs": np.ascontiguousarray(data["srcs"][c]),
            "dstl": np.ascontiguousarray(data["dstl"][c]),
            "iota": data["iota"],
            "discol": np.ascontiguousarray(data["dis_col"][c]),
            "w1": w1_flat, "w2": w2_flat, "b1": b1_in, "b2": b2_in,
        })

    nc = _build_program(cfg, meta)
    res = run_bass_kernel_spmd(nc, in_maps, list(range(ncores)),
                               trace=trace)
    full = np.concatenate([res.results[c]["out"] for c in range(ncores)],
                          axis=0)
    out = np.empty((n, cfg.DO), dtype=np.float32)
    valid = meta["old_of_new"] >= 0
    out[meta["old_of_new"][valid]] = full[valid]
    return out, res


def kernel(x, edge_index, W1, b1, W2, b2):
    out, _ = _run(x, edge_index, W1, b1, W2, b2)
    return out
